# revision 1
# baseline (speedup 1.0000x reference)
"""Trainium2 Bass kernel for nn_MeinBlock (2-layer GCN w/ PReLU + BatchNorm).

Computation (reference):
    h = PReLU(x, a1); h = BN(h, gamma, beta)
    h = GCNConv(h, W1, b1, edges); h = PReLU(h, a2)
    out = GCNConv(h, W2, b2, edges)

GCNConv(h) = dinv * segsum_dst(g[src]) + g*dinv + b   where
    g = dinv * (h @ W),  dinv = deg^-1/2, deg = 1 + indegree.
(The self-loop term (h@W)/deg equals g*dinv.)

Distribution: nodes sharded 8 ways (dst-partitioned edges per the hint).
Each core builds its g-shard, an AllGather forms the full bf16 gather
table, dma_gather fetches messages (grouped by src shard so indices fit
int16), dma_scatter_add (CCE) accumulates into per-core DRAM accumulators.
Duplicate destinations within one scatter call would race in the SDMA CCE,
so edges are split into occurrence rounds (round r = r-th edge of its dst
within its group); rounds alternate between two accumulators.
BN batch stats are combined with a tiny AllReduce.
"""

import os
import sys
from contextlib import ExitStack

import numpy as np

sys.path.insert(0, "/opt/trn_rl_repo")

from concourse import bacc, bass, mybir, tile  # noqa: E402
from concourse import bass_utils as _bu  # noqa: E402
from concourse.bass_utils import run_bass_kernel_spmd  # noqa: E402
from concourse.masks import make_identity  # noqa: E402

# The image's antenv lacks axon_hooks; register the NTFF profile hook
# ourselves so trace=True can report HW exec time. Harmless if it fails.
def _install_ntff_hook():
    import types

    try:
        import antenv.axon_hooks  # noqa: F401
        return
    except ImportError:
        pass
    try:
        import antenv
        from trn_agent_boot.trn_boot import _ntff_profile_via_ctypes

        hook = _ntff_profile_via_ctypes("/opt/axon/libaxon_pjrt.so")
        mod = types.ModuleType("antenv.axon_hooks")
        mod.get_axon_ntff_profile_hook = lambda: hook
        mod.set_axon_ntff_profile_hook = lambda h: None
        sys.modules["antenv.axon_hooks"] = mod
        antenv.axon_hooks = mod
    except Exception:
        pass


_install_ntff_hook()
_bu.upload_artifacts = lambda tmpdir: tmpdir  # no artifact bucket here

F32 = mybir.dt.float32
BF16 = mybir.dt.bfloat16
I16 = mybir.dt.int16

P = 128          # partitions
D = 128          # feature dim
NC = 8           # cores
EPS = 1e-5
DUMMYROWS = 128  # scatter pad target rows appended to each accumulator


def _ceil(a, b):
    return -(-a // b)


def _rup(a, b):
    return _ceil(a, b) * b


# --------------------------------------------------------------------------
# Host-side edge plan (pure index manipulation = the sharding step)
# --------------------------------------------------------------------------
class EdgePlan:
    """Per-core gather/scatter index tensors + static layout metadata."""

    def __init__(self, src, dst, n_nodes):
        shard = n_nodes // NC
        self.shard = shard
        deg = np.bincount(dst, minlength=n_nodes).astype(np.float64) + 1.0
        self.dinv = (1.0 / np.sqrt(deg)).astype(np.float32)

        per_core = []  # (j_arr, r_arr, src_local, dst_local) sorted by (j, r)
        maxlen = np.zeros((NC, 64), dtype=np.int64)  # [j, r] -> max count
        maxr = np.zeros(NC, dtype=np.int64)
        for c in range(NC):
            m = (dst // shard) == c
            es, ed = src[m], dst[m]
            dl = (ed - c * shard).astype(np.int64)
            j = (es // shard).astype(np.int64)
            sl = (es - j * shard).astype(np.int64)
            # occurrence round of each edge's dst within its group j
            order = np.lexsort((dl, j))
            j_s, dl_s, sl_s = j[order], dl[order], sl[order]
            # cumcount within equal (j, dst) runs
            key = j_s * shard + dl_s
            first = np.ones(len(key), dtype=bool)
            first[1:] = key[1:] != key[:-1]
            run_id = np.cumsum(first) - 1
            run_start = np.flatnonzero(first)
            r = np.arange(len(key)) - run_start[run_id]
            # final order: by (j, r)
            order2 = np.lexsort((r, j_s))
            j_f, r_f = j_s[order2], r[order2]
            per_core.append((j_f, r_f, sl_s[order2], dl_s[order2]))
            for jj in range(NC):
                mj = j_f == jj
                if mj.any():
                    rj = r_f[mj]
                    maxr[jj] = max(maxr[jj], rj.max() + 1)
                    cnt = np.bincount(rj)
                    maxlen[jj, : len(cnt)] = np.maximum(maxlen[jj, : len(cnt)], cnt)

        # static padded layout shared by all cores
        self.rounds = []  # per group j: list of (offset, padded_len)
        self.caps = []    # per group j: total padded length
        off = 0
        for jj in range(NC):
            rl = []
            goff = off
            for rr in range(int(maxr[jj])):
                ln = int(_rup(max(int(maxlen[jj, rr]), 1), P))
                rl.append((off - goff, ln))
                off += ln
            self.rounds.append(rl)
            self.caps.append(off - goff)
        self.sumcap = off

        # fill per-core padded index arrays
        self.src16 = np.zeros((NC, P, self.sumcap // 16), dtype=np.int16)
        self.dst16 = np.zeros((NC, P, self.sumcap // 16), dtype=np.int16)
        for c in range(NC):
            j_f, r_f, sl_f, dl_f = per_core[c]
            sarr = np.zeros(self.sumcap, dtype=np.int16)
            darr = np.zeros(self.sumcap, dtype=np.int16)
            goff = 0
            for jj in range(NC):
                for rr, (roff, rlen) in enumerate(self.rounds[jj]):
                    mjr = (j_f == jj) & (r_f == rr)
                    n = int(mjr.sum())
                    assert n <= rlen
                    base = goff + roff
                    sarr[base : base + n] = sl_f[mjr]
                    darr[base : base + n] = dl_f[mjr]
                    npad = rlen - n
                    if npad:
                        sarr[base + n : base + rlen] = 0
                        darr[base + n : base + rlen] = shard + (
                            np.arange(npad) % DUMMYROWS
                        )
                goff += self.caps[jj]
            # wrap: index i -> [i % 16, i // 16], replicated to 128 partitions
            w = sarr.reshape(-1, 16).T
            self.src16[c] = np.tile(w, (8, 1))
            w = darr.reshape(-1, 16).T
            self.dst16[c] = np.tile(w, (8, 1))


# --------------------------------------------------------------------------
# Device program
# --------------------------------------------------------------------------
def build_program(n_nodes, caps, rounds):
    """One SPMD program for all 8 cores. caps/rounds = static edge layout."""
    shard = n_nodes // NC
    shard_pad = _rup(shard, P)
    nt = shard_pad // P                 # 128-node tiles per shard
    sumcap = sum(caps)
    accrows = shard + DUMMYROWS

    nc = bacc.Bacc(
        "TRN2",
        target_bir_lowering=False,
        debug=False,
        num_devices=NC,
        num_swdge_queues=4,
    )

    x_sh = nc.declare_dram_parameter("x_sh", [shard, D], F32, isOutput=False)
    w1 = nc.declare_dram_parameter("w1", [D, D], F32, isOutput=False)
    w2 = nc.declare_dram_parameter("w2", [D, D], F32, isOutput=False)
    b1r = nc.declare_dram_parameter("b1r", [1, D], F32, isOutput=False)
    b2r = nc.declare_dram_parameter("b2r", [1, D], F32, isOutput=False)
    gam = nc.declare_dram_parameter("gam", [D, 1], F32, isOutput=False)
    bet = nc.declare_dram_parameter("bet", [D, 1], F32, isOutput=False)
    a1 = nc.declare_dram_parameter("a1", [1, 1], F32, isOutput=False)
    a2 = nc.declare_dram_parameter("a2", [1, 1], F32, isOutput=False)
    dinv_r = nc.declare_dram_parameter("dinv_r", [1, shard_pad], F32, isOutput=False)
    dinv_c = nc.declare_dram_parameter("dinv_c", [P, nt], F32, isOutput=False)
    src_idx = nc.declare_dram_parameter("src_idx", [P, sumcap // 16], I16, isOutput=False)
    dst_idx = nc.declare_dram_parameter("dst_idx", [P, sumcap // 16], I16, isOutput=False)
    out = nc.declare_dram_parameter("out", [shard, D], F32, isOutput=True)

    g_sh = [nc.dram_tensor(f"g{i}_sh", [shard_pad, D], BF16) for i in (1, 2)]
    g_full = [
        nc.dram_tensor(f"g{i}_full", [n_nodes, D], BF16, addr_space="Shared")
        for i in (1, 2)
    ]
    accs = [
        [nc.dram_tensor(f"acc{i}_{m}", [accrows, D], BF16) for m in range(2)]
        for i in (0, 1)
    ]
    bn_in = nc.dram_tensor("bn_in", [P, 2], F32)
    bn_out = nc.dram_tensor("bn_out", [P, 2], F32, addr_space="Shared")
    dbg_out1 = nc.dram_tensor("dbg_out1", [shard_pad, D], F32)

    CH = 8           # 128-node tiles per big chunk
    CHN = CH * P     # nodes per big chunk (1024)

    with tile.TileContext(nc) as tc, ExitStack() as ctx:
        singles = ctx.enter_context(tc.tile_pool(name="singles", bufs=1))
        big = ctx.enter_context(tc.tile_pool(name="big", bufs=1))
        xin = ctx.enter_context(tc.tile_pool(name="xin", bufs=2))
        work = ctx.enter_context(tc.tile_pool(name="work", bufs=2))
        stream = ctx.enter_context(tc.tile_pool(name="stream", bufs=3))
        gout = ctx.enter_context(tc.tile_pool(name="gout", bufs=3))
        msgs_tp = ctx.enter_context(tc.tile_pool(name="msgs", bufs=8))
        mm_tp = ctx.enter_context(tc.tile_pool(name="mm", bufs=2, space="PSUM"))
        tp_tp = ctx.enter_context(tc.tile_pool(name="tp", bufs=4, space="PSUM"))
        stat_tp = ctx.enter_context(tc.tile_pool(name="stat", bufs=1))

        # ---- constants -------------------------------------------------
        idbf = singles.tile([P, P], BF16)
        make_identity(nc, idbf[:])
        a1c = singles.tile([P, 1], F32)
        nc.sync.dma_start(out=a1c[:], in_=a1[:].to_broadcast([P, 1]))
        a2c = singles.tile([P, 1], F32)
        nc.sync.dma_start(out=a2c[:], in_=a2[:].to_broadcast([P, 1]))
        b1row = singles.tile([P, D], F32)
        nc.sync.dma_start(out=b1row[:], in_=b1r[:].to_broadcast([P, D]))
        b2row = singles.tile([P, D], F32)
        nc.sync.dma_start(out=b2row[:], in_=b2r[:].to_broadcast([P, D]))
        gamc = singles.tile([P, 1], F32)
        nc.sync.dma_start(out=gamc[:], in_=gam[:])
        betc = singles.tile([P, 1], F32)
        nc.sync.dma_start(out=betc[:], in_=bet[:])
        dinvc = singles.tile([P, nt], F32)
        nc.sync.dma_start(out=dinvc[:], in_=dinv_c[:])
        w1f = singles.tile([P, D], F32)
        nc.sync.dma_start(out=w1f[:], in_=w1[:])
        w1b = singles.tile([P, D], BF16)
        nc.vector.tensor_copy(w1b[:], w1f[:])
        w2f = singles.tile([P, D], F32)
        nc.sync.dma_start(out=w2f[:], in_=w2[:])
        w2b = singles.tile([P, D], BF16)
        nc.vector.tensor_copy(w2b[:], w2f[:])
        sidx = singles.tile([P, sumcap // 16], I16)
        nc.sync.dma_start(out=sidx[:], in_=src_idx[:])
        didx = singles.tile([P, sumcap // 16], I16)
        nc.sync.dma_start(out=didx[:], in_=dst_idx[:])
        zt = singles.tile([P, CH // 2, P], BF16)
        nc.vector.memset(zt[:], 0.0)

        hT = big.tile([P, shard_pad], BF16, tag="hbig")

        def zero_acc(acc):
            zn = CH // 2 * P
            for s in range(_ceil(accrows, zn)):
                r0 = s * zn
                rows = min(zn, accrows - r0)
                full, rem = divmod(rows, P)
                if full:
                    dst = acc[r0 : r0 + full * P, :].rearrange(
                        "(t p) f -> p t f", p=P
                    )
                    nc.sync.dma_start(out=dst, in_=zt[:, :full, :])
                if rem:
                    dst2 = acc[r0 + full * P : r0 + rows, :]
                    nc.sync.dma_start(out=dst2, in_=zt[:rem, 0, :])

        def load_node_chunk(dram, r0, rows, dtype, pool):
            """DRAM rows [r0, r0+rows) -> SBUF [128, ceil(rows/128), 128]."""
            full, rem = divmod(rows, P)
            t = pool.tile([P, CH, P], dtype, tag="ld")
            if full:
                src = dram[r0 : r0 + full * P, :].rearrange("(t p) f -> p t f", p=P)
                nc.sync.dma_start(out=t[:, :full, :], in_=src)
            if rem:
                nc.vector.memset(t[:, full, :], 0.0)
                nc.sync.dma_start(
                    out=t[:rem, full, :], in_=dram[r0 + full * P : r0 + rows, :]
                )
            return t

        def transpose_block(src_bf16, ntile, dst_big, col0):
            """node-major [128, ntile, 128] -> dst_big[:, col0 : col0+128*ntile]."""
            for k in range(ntile):
                pt = tp_tp.tile([P, P], BF16, tag="tp")
                nc.tensor.transpose(out=pt[:], in_=src_bf16[:, k, :], identity=idbf[:])
                nc.any.tensor_copy(
                    out=dst_big[:, col0 + k * P : col0 + (k + 1) * P], in_=pt[:]
                )

        def prelu_chunk(x_f32, ac, ntile, out_dtype, pool):
            """max(x, a*x) on [128, ntile, 128]."""
            ax = pool.tile([P, CH, P], F32, tag="ax")
            nc.vector.tensor_scalar_mul(ax[:, :ntile, :], x_f32[:, :ntile, :], ac[:, :1])
            h = pool.tile([P, CH, P], out_dtype, tag="h")
            nc.vector.tensor_tensor(
                out=h[:, :ntile, :],
                in0=x_f32[:, :ntile, :],
                in1=ax[:, :ntile, :],
                op=mybir.AluOpType.max,
            )
            return h

        # ================= conv1 phase A: x -> hT (bf16, feature-major) ====
        nch = _ceil(shard, CHN)
        for s in range(nch):
            r0 = s * CHN
            rows = min(CHN, shard - r0)
            ntile = _ceil(rows, P)
            xt = load_node_chunk(x_sh, r0, rows, F32, xin)
            h = prelu_chunk(xt, a1c, ntile, BF16, work)
            transpose_block(h, ntile, hT, r0)
        if shard_pad > _rup(shard, P):
            nc.vector.memset(hT[:, _rup(shard, P) :], 0.0)

        # ================= BN stats + allreduce ============================
        q = 500 if shard % 500 == 0 else int(np.gcd(shard, 512))
        while shard % q or q > 512:
            q -= 1
        sg = shard // q
        stats = stat_tp.tile([P, sg, 6], F32)
        hT3 = hT[:, :shard].rearrange("p (s q) -> p s q", q=q)
        for i in range(sg):
            nc.vector.bn_stats(out=stats[:, i, :], in_=hT3[:, i, :])
        mv = stat_tp.tile([P, 2], F32)
        nc.vector.bn_aggr(out=mv[:], in_=stats[:])
        # allreduce (mean/8, (var+mean^2)/8)
        ar = stat_tp.tile([P, 2], F32)
        nc.vector.tensor_tensor(
            out=ar[:, 1:2], in0=mv[:, 0:1], in1=mv[:, 0:1], op=mybir.AluOpType.mult
        )
        nc.vector.tensor_tensor(
            out=ar[:, 1:2], in0=ar[:, 1:2], in1=mv[:, 1:2], op=mybir.AluOpType.add
        )
        nc.vector.tensor_scalar_mul(ar[:, 1:2], ar[:, 1:2], 1.0 / NC)
        nc.vector.tensor_scalar_mul(ar[:, 0:1], mv[:, 0:1], 1.0 / NC)
        nc.sync.dma_start(out=bn_in[:], in_=ar[:])
        nc.gpsimd.collective_compute(
            "AllReduce",
            mybir.AluOpType.add,
            replica_groups=[list(range(NC))],
            ins=[bn_in[:]],
            outs=[bn_out[:]],
        )
        st = stat_tp.tile([P, 2], F32)
        nc.sync.dma_start(out=st[:], in_=bn_out[:])
        var = stat_tp.tile([P, 1], F32)
        nc.vector.tensor_tensor(
            out=var[:], in0=st[:, 0:1], in1=st[:, 0:1], op=mybir.AluOpType.mult
        )
        nc.vector.tensor_tensor(
            out=var[:], in0=st[:, 1:2], in1=var[:], op=mybir.AluOpType.subtract
        )
        epst = stat_tp.tile([P, 1], F32)
        nc.vector.memset(epst[:], EPS)
        rstd = stat_tp.tile([P, 1], F32)
        nc.scalar.activation(
            out=rstd[:],
            in_=var[:],
            func=mybir.ActivationFunctionType.Sqrt,
            bias=epst[:],
        )
        nc.vector.reciprocal(out=rstd[:], in_=rstd[:])
        scol = stat_tp.tile([P, 1], F32)
        nc.vector.tensor_tensor(
            out=scol[:], in0=gamc[:], in1=rstd[:], op=mybir.AluOpType.mult
        )
        tcol = stat_tp.tile([P, 1], F32)
        nc.vector.tensor_tensor(
            out=tcol[:], in0=st[:, 0:1], in1=scol[:], op=mybir.AluOpType.mult
        )
        nc.vector.tensor_tensor(
            out=tcol[:], in0=betc[:], in1=tcol[:], op=mybir.AluOpType.subtract
        )

        # ============== shared: hT -> g (normalize? -> dinv -> matmul -> T)
        MC = 512  # nodes per matmul chunk

        def build_g(conv, src_big, wts, g_dst):
            nmc = _ceil(shard_pad, MC)
            for m in range(nmc):
                c0 = m * MC
                cols = min(MC, shard_pad - c0)
                if conv == 1:
                    nh = stream.tile([P, MC], BF16, tag="nh")
                    nc.scalar.activation(
                        out=nh[:, :cols],
                        in_=src_big[:, c0 : c0 + cols],
                        func=mybir.ActivationFunctionType.Identity,
                        bias=tcol[:],
                        scale=scol[:],
                    )
                    base = nh
                else:
                    base = None
                dvb = stream.tile([P, MC], F32, tag="dvb")
                nc.sync.dma_start(
                    out=dvb[:, :cols],
                    in_=dinv_r[0:1, c0 : c0 + cols].to_broadcast([P, cols]),
                )
                dv = stream.tile([P, MC], BF16, tag="dv")
                nc.vector.tensor_tensor(
                    out=dv[:, :cols],
                    in0=(base[:, :cols] if base is not None else src_big[:, c0 : c0 + cols]),
                    in1=dvb[:, :cols],
                    op=mybir.AluOpType.mult,
                )
                mm = mm_tp.tile([P, MC], F32, tag="mm")
                nc.tensor.matmul(
                    out=mm[:, :cols], lhsT=wts[:], rhs=dv[:, :cols], start=True, stop=True
                )
                gT = stream.tile([P, MC], BF16, tag="gT")
                nc.any.tensor_copy(out=gT[:, :cols], in_=mm[:, :cols])
                stg = gout.tile([P, MC // P, P], BF16, tag="stg")
                for k in range(_ceil(cols, P)):
                    pt = tp_tp.tile([P, P], BF16, tag="tp")
                    nc.tensor.transpose(
                        out=pt[:], in_=gT[:, k * P : (k + 1) * P], identity=idbf[:]
                    )
                    nc.any.tensor_copy(out=stg[:, k, :], in_=pt[:])
                rows0 = c0
                rows = min(MC, shard_pad - rows0)
                dst = g_dst[rows0 : rows0 + rows, :].rearrange("(t p) f -> p t f", p=P)
                nc.sync.dma_start(out=dst, in_=stg[:, : rows // P, :])

        # ============== edge phase: gather + scatter rounds ================
        # SWDGE ring holds 1024 descriptors -> <=1024-index calls. Tile hands
        # the 8 DMASW sem lanes to SWDGE ops round-robin in Pool program
        # order, and each sem is locked to one queue; queue_num = (k%8)//2
        # keeps the sem<->queue binding consistent while spreading calls over
        # all 4 rings (SDMA drains rings round-robin -> parallel drain).
        GCH = 1024

        def swq():
            return 0  # rewritten post-scheduling from the assigned DMASW lane

        def edge_phase(g_full_t, acc_pair):
            goff = 0
            sctr = 0
            for j in range(NC):
                cap = caps[j]
                if cap == 0:
                    continue
                # scatter split points: round boundaries + chunk boundaries
                bounds = sorted(
                    {0, cap}
                    | {roff for roff, _ in rounds[j]}
                    | {c for c in range(GCH, cap, GCH)}
                )
                for c0 in range(0, cap, GCH):
                    clen = min(GCH, cap - c0)
                    msgs = msgs_tp.tile([P, GCH // P, P], BF16, tag="msgs")
                    nc.gpsimd.dma_gather(
                        msgs[:, : clen // P, :],
                        g_full_t[j * shard : (j + 1) * shard, :],
                        sidx[:, (goff + c0) // 16 : (goff + c0 + clen) // 16],
                        clen,
                        clen,
                        D,
                        queue_num=swq(),
                        single_packet=True,
                    )
                    subs = [b for b in bounds if c0 <= b <= c0 + clen]
                    for a, b in zip(subs, subs[1:]):
                        acc = acc_pair[sctr % len(acc_pair)]
                        sctr += 1
                        nc.gpsimd.dma_scatter_add(
                            acc[:],
                            msgs[:, (a - c0) // P : (b - c0) // P, :],
                            didx[:, (goff + a) // 16 : (goff + b) // 16],
                            b - a,
                            b - a,
                            D,
                            queue_num=swq(),
                            single_packet=True,
                        )
                goff += cap

        # ============== readback: out_nm = dinv*(acc0+acc1+g_own) + brow ===
        def readback(acc_pair, g_own, brow, store_out, prelu_a, dst_big):
            for s in range(nch):
                r0 = s * CHN
                rows = min(CHN, shard - r0)
                ntile = _ceil(rows, P)
                at0 = xin.tile([P, CH, P], BF16, tag="at0")
                src = acc_pair[0][r0 : r0 + ntile * P, :].rearrange(
                    "(t p) f -> p t f", p=P
                )
                nc.sync.dma_start(out=at0[:, :ntile, :], in_=src)
                at1 = xin.tile([P, CH, P], BF16, tag="at1")
                src = acc_pair[1][r0 : r0 + ntile * P, :].rearrange(
                    "(t p) f -> p t f", p=P
                )
                nc.sync.dma_start(out=at1[:, :ntile, :], in_=src)
                gt = xin.tile([P, CH, P], BF16, tag="gt")
                src = g_own[r0 : r0 + ntile * P, :].rearrange("(t p) f -> p t f", p=P)
                nc.sync.dma_start(out=gt[:, :ntile, :], in_=src)

                sm = work.tile([P, CH, P], F32, tag="sm")
                nc.vector.tensor_tensor(
                    out=sm[:, :ntile, :],
                    in0=at0[:, :ntile, :],
                    in1=at1[:, :ntile, :],
                    op=mybir.AluOpType.add,
                )
                nc.vector.tensor_tensor(
                    out=sm[:, :ntile, :],
                    in0=sm[:, :ntile, :],
                    in1=gt[:, :ntile, :],
                    op=mybir.AluOpType.add,
                )
                # * dinv (per-node = per (partition, tile)) via stride-0 f bcast
                dv_ap = bass.AP(
                    tensor=dinvc.tensor,
                    offset=dinvc.offset + s * CH,
                    ap=[list(dinvc.ap[0]), [1, ntile], [0, P]],
                )
                nc.vector.tensor_tensor(
                    out=sm[:, :ntile, :],
                    in0=sm[:, :ntile, :],
                    in1=dv_ap,
                    op=mybir.AluOpType.mult,
                )
                # + b row (replicated tile; bcast over the tile dim only)
                br_ap = bass.AP(
                    tensor=brow.tensor,
                    offset=brow.offset,
                    ap=[list(brow.ap[0]), [0, ntile], [1, P]],
                )
                ot = sm
                nc.vector.tensor_tensor(
                    out=ot[:, :ntile, :],
                    in0=sm[:, :ntile, :],
                    in1=br_ap,
                    op=mybir.AluOpType.add,
                )
                if store_out:
                    full, rem = divmod(rows, P)
                    if full:
                        dst = out[r0 : r0 + full * P, :].rearrange(
                            "(t p) f -> p t f", p=P
                        )
                        nc.sync.dma_start(out=dst, in_=ot[:, :full, :])
                    if rem:
                        nc.sync.dma_start(
                            out=out[r0 + full * P : r0 + rows, :],
                            in_=ot[:rem, full, :],
                        )
                else:
                    dbg_dst = dbg_out1[r0 : r0 + ntile * P, :].rearrange(
                        "(t p) f -> p t f", p=P
                    )
                    nc.sync.dma_start(out=dbg_dst, in_=ot[:, :ntile, :])
                    h2 = prelu_chunk(ot, prelu_a, ntile, BF16, work)
                    transpose_block(h2, ntile, dst_big, r0)

        # =================== schedule both convs ===========================
        for m in range(2):
            zero_acc(accs[0][m])
        build_g(1, hT, w1b, g_sh[0])
        nc.gpsimd.collective_compute(
            "AllGather",
            mybir.AluOpType.bypass,
            replica_groups=[list(range(NC))],
            ins=[g_sh[0][:shard, :]],
            outs=[g_full[0][:]],
        )
        edge_phase(g_full[0], accs[0])
        h2T = big.tile([P, shard_pad], BF16, tag="hbig")
        readback(accs[0], g_sh[0], b1row, False, a2c, h2T)
        if shard_pad > _rup(shard, P):
            nc.vector.memset(h2T[:, _rup(shard, P) :], 0.0)

        for m in range(2):
            zero_acc(accs[1][m])
        build_g(2, h2T, w2b, g_sh[1])
        nc.gpsimd.collective_compute(
            "AllGather",
            mybir.AluOpType.bypass,
            replica_groups=[list(range(NC))],
            ins=[g_sh[1][:shard, :]],
            outs=[g_full[1][:]],
        )
        edge_phase(g_full[1], accs[1])
        readback(accs[1], g_sh[1], b2row, True, None, None)

    # Spread SWDGE calls over the 4 rings, consistent with Tile's DMASW sem
    # lane assignment (lane k <-> queue k//2) so each sem stays locked to
    # one queue while the rings drain in parallel.
    from concourse.tile_sem_assignment import PROC_NAME_TO_IDX

    lane_of = {PROC_NAME_TO_IDX[f"DMASW{k}"]: k for k in range(8)}
    for inst in nc.inst_map.values():
        if isinstance(inst, (mybir.InstDMAGatherAnt, mybir.InstDMAScatterAddAnt)):
            proc = getattr(inst, "bass_scheduled_proc", None)
            if proc in lane_of:
                inst.queue_num = lane_of[proc] // 2

    nc.compile()
    return nc


# --------------------------------------------------------------------------
# Host wrapper
# --------------------------------------------------------------------------
def _prep_inputs(x, edge_index, a1, gamma, beta, W1, b1, a2, W2, b2):
    n = x.shape[0]
    shard = n // NC
    shard_pad = _rup(shard, P)
    nt = shard_pad // P
    ei = np.asarray(edge_index).astype(np.int64)
    plan = EdgePlan(ei[0], ei[1], n)

    in_maps = []
    for c in range(NC):
        dv = plan.dinv[c * shard : (c + 1) * shard]
        dinv_r = np.zeros((1, shard_pad), dtype=np.float32)
        dinv_r[0, :shard] = dv
        dvp = np.zeros(shard_pad, dtype=np.float32)
        dvp[:shard] = dv
        dinv_c = np.ascontiguousarray(dvp.reshape(nt, P).T)  # [p,t]=dinv[t*128+p]
        in_maps.append(
            dict(
                x_sh=np.ascontiguousarray(x[c * shard : (c + 1) * shard]).astype(
                    np.float32
                ),
                w1=np.asarray(W1, dtype=np.float32),
                w2=np.asarray(W2, dtype=np.float32),
                b1r=np.asarray(b1, dtype=np.float32).reshape(1, D),
                b2r=np.asarray(b2, dtype=np.float32).reshape(1, D),
                gam=np.asarray(gamma, dtype=np.float32).reshape(D, 1),
                bet=np.asarray(beta, dtype=np.float32).reshape(D, 1),
                a1=np.asarray(a1, dtype=np.float32).reshape(1, 1),
                a2=np.asarray(a2, dtype=np.float32).reshape(1, 1),
                dinv_r=dinv_r,
                dinv_c=dinv_c,
                src_idx=plan.src16[c],
                dst_idx=plan.dst16[c],
            )
        )
    return plan, in_maps


_PROG_CACHE = {}


def kernel(x, edge_index, a1, gamma, beta, W1, b1, a2, W2, b2, _trace=False):
    x = np.asarray(x)
    n = x.shape[0]
    plan, in_maps = _prep_inputs(
        x, edge_index, a1, gamma, beta, W1, b1, a2, W2, b2
    )
    key = (n, tuple(plan.caps), tuple(tuple(r) for r in plan.rounds))
    if key not in _PROG_CACHE:
        _PROG_CACHE[key] = build_program(n, plan.caps, plan.rounds)
    nc = _PROG_CACHE[key]
    res = run_bass_kernel_spmd(
        nc, in_maps, core_ids=list(range(NC)), trace=_trace
    )
    outs = [res.results[c]["out"] for c in range(NC)]
    full = np.concatenate(outs, axis=0).astype(np.float32)
    kernel._last_exec_ns = res.exec_time_ns
    return full



# revision 5
# speedup vs baseline: 1.1496x; 1.1496x over previous
"""Trainium2 Bass kernel for nn_MeinBlock (2-layer GCN w/ PReLU + BatchNorm).

Computation (reference):
    h = PReLU(x, a1); h = BN(h, gamma, beta)
    h = GCNConv(h, W1, b1, edges); h = PReLU(h, a2)
    out = GCNConv(h, W2, b2, edges)

GCNConv(h) = dinv * segsum_dst(g[src]) + g*dinv + b   where
    g = dinv * (h @ W),  dinv = deg^-1/2, deg = 1 + indegree.
(The self-loop term (h@W)/deg equals g*dinv.)

Distribution: nodes sharded 8 ways (dst-partitioned edges per the hint).
Each core builds its g-shard in 8 row-chunks of 3200 (g_shc tensors); 8
chunked AllGathers (one per row-chunk) each produce a 25600-row bf16
gather table, so edge processing for chunk c starts as soon as collective
c lands -- the collective pipeline overlaps the gather/scatter pipeline.
Edges are grouped by the source row-chunk c = (src % shard) // 3200 and
dma_gather fetches messages (row index = src_shard*3200 + offset fits
int16), dma_scatter_add (CCE) accumulates into per-core DRAM accumulators.
Duplicate destinations within one scatter call would race in the SDMA CCE,
so edges are split into occurrence rounds (round r = r-th edge of its dst
within its chunk group); rounds alternate between two accumulators.
SWDGE calls are 512 indices and adjacent calls go to different rings
(queue = DMASW lane % 4) so all four rings drain concurrently instead of
head-of-line blocking on a single full ring.
BN batch stats are combined with a tiny AllReduce.
"""

import os
import sys
from contextlib import ExitStack

import numpy as np

sys.path.insert(0, "/opt/trn_rl_repo")

from concourse import bacc, bass, mybir, tile  # noqa: E402
from concourse import bass_utils as _bu  # noqa: E402
from concourse.bass_utils import run_bass_kernel_spmd  # noqa: E402
from concourse.masks import make_identity  # noqa: E402

# The image's antenv lacks axon_hooks; register the NTFF profile hook
# ourselves so trace=True can report HW exec time. Harmless if it fails.
def _install_ntff_hook():
    import types

    try:
        import antenv.axon_hooks  # noqa: F401
        return
    except ImportError:
        pass
    try:
        import antenv
        from trn_agent_boot.trn_boot import _ntff_profile_via_ctypes

        hook = _ntff_profile_via_ctypes("/opt/axon/libaxon_pjrt.so")
        mod = types.ModuleType("antenv.axon_hooks")
        mod.get_axon_ntff_profile_hook = lambda: hook
        mod.set_axon_ntff_profile_hook = lambda h: None
        sys.modules["antenv.axon_hooks"] = mod
        antenv.axon_hooks = mod
    except Exception:
        pass


_install_ntff_hook()
_bu.upload_artifacts = lambda tmpdir: tmpdir  # no artifact bucket here

F32 = mybir.dt.float32
BF16 = mybir.dt.bfloat16
I16 = mybir.dt.int16

P = 128          # partitions
D = 128          # feature dim
NC = 8           # cores
NCHK = 8         # collective row-chunks per shard
EPS = 1e-5
DUMMYROWS = 128  # scatter pad target rows appended to each accumulator
GCH = 512        # indices per SWDGE call (ring holds 1024 descs -> 2 calls)


def _ceil(a, b):
    return -(-a // b)


def _rup(a, b):
    return _ceil(a, b) * b


# --------------------------------------------------------------------------
# Host-side edge plan (pure index manipulation = the sharding step)
# --------------------------------------------------------------------------
class EdgePlan:
    """Per-core gather/scatter index tensors + static layout metadata.

    Edges of core c (dst in its shard) are grouped by the SOURCE row-chunk
    g = (src % shard) // chk (chk = shard_pad // NCHK rows per collective
    chunk).  Gather table for group g is the AllGather of every shard's
    rows [g*chk, (g+1)*chk): table row = src_shard * chk + (src_local -
    g*chk) < NC*chk = shard_pad <= 32767 (int16).
    """

    def __init__(self, src, dst, n_nodes):
        shard = n_nodes // NC
        self.shard = shard
        shard_pad = _rup(shard, NCHK * P)
        chk = shard_pad // NCHK
        assert chk * NCHK == shard_pad and NC * chk < 32768
        self.chk = chk
        deg = np.bincount(dst, minlength=n_nodes).astype(np.float64) + 1.0
        self.dinv = (1.0 / np.sqrt(deg)).astype(np.float32)

        per_core = []  # (g_arr, r_arr, tblrow, dst_local) sorted by (g, r)
        maxlen = np.zeros((NCHK, 64), dtype=np.int64)  # [g, r] -> max count
        maxr = np.zeros(NCHK, dtype=np.int64)
        for c in range(NC):
            m = (dst // shard) == c
            es, ed = src[m], dst[m]
            dl = (ed - c * shard).astype(np.int64)
            j = (es // shard).astype(np.int64)
            sl = (es - j * shard).astype(np.int64)   # local row in src shard
            g = sl // chk                            # source row-chunk group
            row = j * chk + (sl - g * chk)           # row in group-g table
            # occurrence round of each edge's dst within its group g
            order = np.lexsort((dl, g))
            g_s, dl_s, row_s = g[order], dl[order], row[order]
            key = g_s * shard + dl_s
            first = np.ones(len(key), dtype=bool)
            first[1:] = key[1:] != key[:-1]
            run_id = np.cumsum(first) - 1
            run_start = np.flatnonzero(first)
            r = np.arange(len(key)) - run_start[run_id]
            # final order: by (g, r)
            order2 = np.lexsort((r, g_s))
            g_f, r_f = g_s[order2], r[order2]
            per_core.append((g_f, r_f, row_s[order2], dl_s[order2]))
            for gg in range(NCHK):
                mg = g_f == gg
                if mg.any():
                    rg = r_f[mg]
                    maxr[gg] = max(maxr[gg], rg.max() + 1)
                    cnt = np.bincount(rg)
                    maxlen[gg, : len(cnt)] = np.maximum(maxlen[gg, : len(cnt)], cnt)

        # static padded layout shared by all cores
        self.rounds = []  # per group g: list of (offset, padded_len)
        self.caps = []    # per group g: total padded length
        off = 0
        for gg in range(NCHK):
            rl = []
            goff = off
            for rr in range(int(maxr[gg])):
                ln = int(_rup(max(int(maxlen[gg, rr]), 1), P))
                rl.append((off - goff, ln))
                off += ln
            self.rounds.append(rl)
            self.caps.append(off - goff)
        self.sumcap = off

        # fill per-core padded index arrays
        self.src16 = np.zeros((NC, P, self.sumcap // 16), dtype=np.int16)
        self.dst16 = np.zeros((NC, P, self.sumcap // 16), dtype=np.int16)
        for c in range(NC):
            g_f, r_f, row_f, dl_f = per_core[c]
            sarr = np.zeros(self.sumcap, dtype=np.int16)
            darr = np.zeros(self.sumcap, dtype=np.int16)
            goff = 0
            for gg in range(NCHK):
                for rr, (roff, rlen) in enumerate(self.rounds[gg]):
                    mgr = (g_f == gg) & (r_f == rr)
                    n = int(mgr.sum())
                    assert n <= rlen
                    base = goff + roff
                    sarr[base : base + n] = row_f[mgr]
                    darr[base : base + n] = dl_f[mgr]
                    npad = rlen - n
                    if npad:
                        sarr[base + n : base + rlen] = 0
                        darr[base + n : base + rlen] = shard + (
                            np.arange(npad) % DUMMYROWS
                        )
                goff += self.caps[gg]
            # wrap: index i -> [i % 16, i // 16], replicated to 128 partitions
            w = sarr.reshape(-1, 16).T
            self.src16[c] = np.tile(w, (8, 1))
            w = darr.reshape(-1, 16).T
            self.dst16[c] = np.tile(w, (8, 1))


# --------------------------------------------------------------------------
# Device program
# --------------------------------------------------------------------------
def build_program(n_nodes, caps, rounds):
    """One SPMD program for all 8 cores. caps/rounds = static edge layout."""
    shard = n_nodes // NC
    shard_pad = _rup(shard, NCHK * P)
    nt = shard_pad // P                 # 128-node tiles per shard
    chk = shard_pad // NCHK             # rows per collective chunk
    tbl = NC * chk                      # rows per gather table
    sumcap = sum(caps)
    accrows = shard + DUMMYROWS

    nc = bacc.Bacc(
        "TRN2",
        target_bir_lowering=False,
        debug=False,
        num_devices=NC,
        num_swdge_queues=4,
    )

    x_sh = nc.declare_dram_parameter("x_sh", [shard, D], F32, isOutput=False)
    w1 = nc.declare_dram_parameter("w1", [D, D], F32, isOutput=False)
    w2 = nc.declare_dram_parameter("w2", [D, D], F32, isOutput=False)
    b1r = nc.declare_dram_parameter("b1r", [1, D], F32, isOutput=False)
    b2r = nc.declare_dram_parameter("b2r", [1, D], F32, isOutput=False)
    gam = nc.declare_dram_parameter("gam", [D, 1], F32, isOutput=False)
    bet = nc.declare_dram_parameter("bet", [D, 1], F32, isOutput=False)
    a1 = nc.declare_dram_parameter("a1", [1, 1], F32, isOutput=False)
    a2 = nc.declare_dram_parameter("a2", [1, 1], F32, isOutput=False)
    dinv_c = nc.declare_dram_parameter("dinv_c", [P, nt], F32, isOutput=False)
    src_idx = nc.declare_dram_parameter("src_idx", [P, sumcap // 16], I16, isOutput=False)
    dst_idx = nc.declare_dram_parameter("dst_idx", [P, sumcap // 16], I16, isOutput=False)
    out = nc.declare_dram_parameter("out", [shard, D], F32, isOutput=True)

    # per-conv: 8 local g row-chunks + 8 allgathered tables
    g_shc = [
        [nc.dram_tensor(f"g{i}_shc{c}", [chk, D], BF16) for c in range(NCHK)]
        for i in (1, 2)
    ]
    g_full = [
        [
            nc.dram_tensor(f"g{i}_full{c}", [tbl, D], BF16, addr_space="Shared")
            for c in range(NCHK)
        ]
        for i in (1, 2)
    ]
    accs = [
        [nc.dram_tensor(f"acc{i}_{m}", [accrows, D], BF16) for m in range(2)]
        for i in (0, 1)
    ]
    bn_in = nc.dram_tensor("bn_in", [P, 2], F32)
    bn_out = nc.dram_tensor("bn_out", [P, 2], F32, addr_space="Shared")

    CH = 8           # 128-node tiles per big chunk
    CHN = CH * P     # nodes per big chunk (1024)

    with tile.TileContext(nc) as tc, ExitStack() as ctx:
        singles = ctx.enter_context(tc.tile_pool(name="singles", bufs=1))
        big = ctx.enter_context(tc.tile_pool(name="big", bufs=1))
        xin = ctx.enter_context(tc.tile_pool(name="xin", bufs=2))
        work = ctx.enter_context(tc.tile_pool(name="work", bufs=2))
        stream = ctx.enter_context(tc.tile_pool(name="stream", bufs=3))
        gout = ctx.enter_context(tc.tile_pool(name="gout", bufs=3))
        msgs_tp = ctx.enter_context(tc.tile_pool(name="msgs", bufs=8))
        mm_tp = ctx.enter_context(tc.tile_pool(name="mm", bufs=2, space="PSUM"))
        tp_tp = ctx.enter_context(tc.tile_pool(name="tp", bufs=4, space="PSUM"))
        stat_tp = ctx.enter_context(tc.tile_pool(name="stat", bufs=1))

        # ---- constants -------------------------------------------------
        idbf = singles.tile([P, P], BF16)
        make_identity(nc, idbf[:])
        a1c = singles.tile([P, 1], F32)
        nc.sync.dma_start(out=a1c[:], in_=a1[:].to_broadcast([P, 1]))
        a2c = singles.tile([P, 1], F32)
        nc.sync.dma_start(out=a2c[:], in_=a2[:].to_broadcast([P, 1]))
        b1row = singles.tile([P, D], F32)
        nc.sync.dma_start(out=b1row[:], in_=b1r[:].to_broadcast([P, D]))
        b2row = singles.tile([P, D], F32)
        nc.sync.dma_start(out=b2row[:], in_=b2r[:].to_broadcast([P, D]))
        gamc = singles.tile([P, 1], F32)
        nc.sync.dma_start(out=gamc[:], in_=gam[:])
        betc = singles.tile([P, 1], F32)
        nc.sync.dma_start(out=betc[:], in_=bet[:])
        dinvc = singles.tile([P, nt], F32)
        nc.sync.dma_start(out=dinvc[:], in_=dinv_c[:])
        w1f = singles.tile([P, D], F32)
        nc.sync.dma_start(out=w1f[:], in_=w1[:])
        w1b = singles.tile([P, D], BF16)
        nc.vector.tensor_copy(w1b[:], w1f[:])
        w2f = singles.tile([P, D], F32)
        nc.sync.dma_start(out=w2f[:], in_=w2[:])
        w2b = singles.tile([P, D], BF16)
        nc.vector.tensor_copy(w2b[:], w2f[:])
        sidx = singles.tile([P, sumcap // 16], I16)
        nc.sync.dma_start(out=sidx[:], in_=src_idx[:])
        didx = singles.tile([P, sumcap // 16], I16)
        nc.sync.dma_start(out=didx[:], in_=dst_idx[:])
        zt = singles.tile([P, CH, P], BF16)
        nc.vector.memset(zt[:], 0.0)

        hT = big.tile([P, shard_pad], BF16, tag="hbig")

        def zero_acc(acc):
            zn = CH * P
            for s in range(_ceil(accrows, zn)):
                r0 = s * zn
                rows = min(zn, accrows - r0)
                full, rem = divmod(rows, P)
                if full:
                    dst = acc[r0 : r0 + full * P, :].rearrange(
                        "(t p) f -> p t f", p=P
                    )
                    nc.sync.dma_start(out=dst, in_=zt[:, :full, :])
                if rem:
                    dst2 = acc[r0 + full * P : r0 + rows, :]
                    nc.sync.dma_start(out=dst2, in_=zt[:rem, 0, :])

        def load_node_chunk(dram, r0, rows, dtype, pool):
            """DRAM rows [r0, r0+rows) -> SBUF [128, ceil(rows/128), 128]."""
            full, rem = divmod(rows, P)
            t = pool.tile([P, CH, P], dtype, tag="ld")
            if full:
                src = dram[r0 : r0 + full * P, :].rearrange("(t p) f -> p t f", p=P)
                nc.sync.dma_start(out=t[:, :full, :], in_=src)
            if rem:
                nc.vector.memset(t[:, full, :], 0.0)
                nc.sync.dma_start(
                    out=t[:rem, full, :], in_=dram[r0 + full * P : r0 + rows, :]
                )
            return t

        def load_chunked_rows(tensors, rowsz, r0, ntile, pool, tag):
            """Rows [r0, r0+ntile*128) from a list of [rowsz, D] DRAM tensors
            (concatenated view) -> SBUF [128, ntile, 128]."""
            t = pool.tile([P, CH, P], BF16, tag=tag)
            a = r0
            while a < r0 + ntile * P:
                c = a // rowsz
                b = min(r0 + ntile * P, (c + 1) * rowsz)
                src = tensors[c][a - c * rowsz : b - c * rowsz, :].rearrange(
                    "(t p) f -> p t f", p=P
                )
                nc.sync.dma_start(
                    out=t[:, (a - r0) // P : (b - r0) // P, :], in_=src
                )
                a = b
            return t

        def transpose_block(src_bf16, ntile, dst_big, col0):
            """node-major [128, ntile, 128] -> dst_big[:, col0 : col0+128*ntile]."""
            for k in range(ntile):
                pt = tp_tp.tile([P, P], BF16, tag="tp")
                nc.tensor.transpose(out=pt[:], in_=src_bf16[:, k, :], identity=idbf[:])
                nc.any.tensor_copy(
                    out=dst_big[:, col0 + k * P : col0 + (k + 1) * P], in_=pt[:]
                )

        def prelu_chunk(x_f32, ac, ntile, out_dtype, pool):
            """max(x, a*x) on [128, ntile, 128]."""
            ax = pool.tile([P, CH, P], F32, tag="ax")
            nc.vector.tensor_scalar_mul(ax[:, :ntile, :], x_f32[:, :ntile, :], ac[:, :1])
            h = pool.tile([P, CH, P], out_dtype, tag="h")
            nc.vector.tensor_tensor(
                out=h[:, :ntile, :],
                in0=x_f32[:, :ntile, :],
                in1=ax[:, :ntile, :],
                op=mybir.AluOpType.max,
            )
            return h

        # ================= conv1 phase A: x -> hT (bf16, feature-major) ====
        nch = _ceil(shard, CHN)
        for s in range(nch):
            r0 = s * CHN
            rows = min(CHN, shard - r0)
            ntile = _ceil(rows, P)
            xt = load_node_chunk(x_sh, r0, rows, F32, xin)
            h = prelu_chunk(xt, a1c, ntile, BF16, work)
            transpose_block(h, ntile, hT, r0)
        if shard_pad > _rup(shard, P):
            nc.vector.memset(hT[:, _rup(shard, P) :], 0.0)

        # ================= BN stats + allreduce ============================
        q = 500 if shard % 500 == 0 else int(np.gcd(shard, 512))
        while shard % q or q > 512:
            q -= 1
        sg = shard // q
        stats = stat_tp.tile([P, sg, 6], F32)
        hT3 = hT[:, :shard].rearrange("p (s q) -> p s q", q=q)
        for i in range(sg):
            nc.vector.bn_stats(out=stats[:, i, :], in_=hT3[:, i, :])
        mv = stat_tp.tile([P, 2], F32)
        nc.vector.bn_aggr(out=mv[:], in_=stats[:])
        # allreduce (mean/8, (var+mean^2)/8)
        ar = stat_tp.tile([P, 2], F32)
        nc.vector.tensor_tensor(
            out=ar[:, 1:2], in0=mv[:, 0:1], in1=mv[:, 0:1], op=mybir.AluOpType.mult
        )
        nc.vector.tensor_tensor(
            out=ar[:, 1:2], in0=ar[:, 1:2], in1=mv[:, 1:2], op=mybir.AluOpType.add
        )
        nc.vector.tensor_scalar_mul(ar[:, 1:2], ar[:, 1:2], 1.0 / NC)
        nc.vector.tensor_scalar_mul(ar[:, 0:1], mv[:, 0:1], 1.0 / NC)
        nc.sync.dma_start(out=bn_in[:], in_=ar[:])
        nc.gpsimd.collective_compute(
            "AllReduce",
            mybir.AluOpType.add,
            replica_groups=[list(range(NC))],
            ins=[bn_in[:]],
            outs=[bn_out[:]],
        )
        st = stat_tp.tile([P, 2], F32)
        nc.sync.dma_start(out=st[:], in_=bn_out[:])
        var = stat_tp.tile([P, 1], F32)
        nc.vector.tensor_tensor(
            out=var[:], in0=st[:, 0:1], in1=st[:, 0:1], op=mybir.AluOpType.mult
        )
        nc.vector.tensor_tensor(
            out=var[:], in0=st[:, 1:2], in1=var[:], op=mybir.AluOpType.subtract
        )
        epst = stat_tp.tile([P, 1], F32)
        nc.vector.memset(epst[:], EPS)
        rstd = stat_tp.tile([P, 1], F32)
        nc.scalar.activation(
            out=rstd[:],
            in_=var[:],
            func=mybir.ActivationFunctionType.Sqrt,
            bias=epst[:],
        )
        nc.vector.reciprocal(out=rstd[:], in_=rstd[:])
        scol = stat_tp.tile([P, 1], F32)
        nc.vector.tensor_tensor(
            out=scol[:], in0=gamc[:], in1=rstd[:], op=mybir.AluOpType.mult
        )
        tcol = stat_tp.tile([P, 1], F32)
        nc.vector.tensor_tensor(
            out=tcol[:], in0=st[:, 0:1], in1=scol[:], op=mybir.AluOpType.mult
        )
        nc.vector.tensor_tensor(
            out=tcol[:], in0=betc[:], in1=tcol[:], op=mybir.AluOpType.subtract
        )

        # ====== shared: hT -> g (normalize? -> matmul -> T -> dinv -> store)
        # Chunked-store into the 8 per-collective g_shc tensors; the matching
        # AllGather is issued as soon as its chunk's rows are all stored, so
        # collectives pipeline with the rest of build_g (and then with the
        # edge phase).
        MC = 512  # nodes per matmul chunk

        def build_g(conv, src_big, wts, g_dst_list):
            nmc = _ceil(shard_pad, MC)
            next_ag = 0
            for m in range(nmc):
                c0 = m * MC
                cols = min(MC, shard_pad - c0)
                if conv == 1:
                    nh = stream.tile([P, MC], BF16, tag="nh")
                    nc.scalar.activation(
                        out=nh[:, :cols],
                        in_=src_big[:, c0 : c0 + cols],
                        func=mybir.ActivationFunctionType.Identity,
                        bias=tcol[:],
                        scale=scol[:],
                    )
                    rhs = nh[:, :cols]
                else:
                    rhs = src_big[:, c0 : c0 + cols]
                mm = mm_tp.tile([P, MC], F32, tag="mm")
                nc.tensor.matmul(
                    out=mm[:, :cols], lhsT=wts[:], rhs=rhs, start=True, stop=True
                )
                gT = stream.tile([P, MC], BF16, tag="gT")
                nc.any.tensor_copy(out=gT[:, :cols], in_=mm[:, :cols])
                stg = gout.tile([P, MC // P, P], BF16, tag="stg")
                for k in range(_ceil(cols, P)):
                    pt = tp_tp.tile([P, P], BF16, tag="tp")
                    nc.tensor.transpose(
                        out=pt[:], in_=gT[:, k * P : (k + 1) * P], identity=idbf[:]
                    )
                    nc.any.tensor_copy(out=stg[:, k, :], in_=pt[:])
                # * dinv (per-node = per (partition, tile)) via stride-0 f bcast
                ntile = _ceil(cols, P)
                dv_ap = bass.AP(
                    tensor=dinvc.tensor,
                    offset=dinvc.offset + c0 // P,
                    ap=[list(dinvc.ap[0]), [1, ntile], [0, P]],
                )
                nc.vector.tensor_tensor(
                    out=stg[:, :ntile, :],
                    in0=stg[:, :ntile, :],
                    in1=dv_ap,
                    op=mybir.AluOpType.mult,
                )
                # store rows [c0, c0+cols) split at chunk boundaries
                a = c0
                while a < c0 + cols:
                    cc = a // chk
                    b = min(c0 + cols, (cc + 1) * chk)
                    dst = g_dst_list[cc][a - cc * chk : b - cc * chk, :].rearrange(
                        "(t p) f -> p t f", p=P
                    )
                    nc.sync.dma_start(
                        out=dst, in_=stg[:, (a - c0) // P : (b - c0) // P, :]
                    )
                    a = b
                # issue any AllGather whose input chunk is now fully stored
                while next_ag < NCHK and (next_ag + 1) * chk <= c0 + cols:
                    yield next_ag
                    next_ag += 1
            while next_ag < NCHK:
                yield next_ag
                next_ag += 1

        def issue_ag(conv_i, c):
            nc.gpsimd.collective_compute(
                "AllGather",
                mybir.AluOpType.bypass,
                replica_groups=[list(range(NC))],
                ins=[g_shc[conv_i][c][:]],
                outs=[g_full[conv_i][c][:]],
            )

        # ============== edge phase: gather + scatter rounds ================
        # SWDGE ring holds 1024 descriptors; GCH=512 so two calls fit per
        # ring. Tile hands the 8 DMASW sem lanes to SWDGE ops round-robin in
        # Pool program order; queue_num is rewritten post-scheduling to
        # lane % 4 so adjacent calls land on different rings and all four
        # rings drain in parallel (lane -> queue stays a static binding).
        def swq():
            return 0  # rewritten post-scheduling from the assigned DMASW lane

        def edge_group(g_full_t, acc_pair, gg, goff, sctr):
            cap = caps[gg]
            bounds = sorted(
                {0, cap}
                | {roff for roff, _ in rounds[gg]}
                | {c for c in range(GCH, cap, GCH)}
            )
            for c0 in range(0, cap, GCH):
                clen = min(GCH, cap - c0)
                msgs = msgs_tp.tile([P, GCH // P, P], BF16, tag="msgs")
                nc.gpsimd.dma_gather(
                    msgs[:, : clen // P, :],
                    g_full_t[:],
                    sidx[:, (goff + c0) // 16 : (goff + c0 + clen) // 16],
                    clen,
                    clen,
                    D,
                    queue_num=swq(),
                    single_packet=True,
                )
                subs = [b for b in bounds if c0 <= b <= c0 + clen]
                for a, b in zip(subs, subs[1:]):
                    acc = acc_pair[sctr[0] % len(acc_pair)]
                    sctr[0] += 1
                    nc.gpsimd.dma_scatter_add(
                        acc[:],
                        msgs[:, (a - c0) // P : (b - c0) // P, :],
                        didx[:, (goff + a) // 16 : (goff + b) // 16],
                        b - a,
                        b - a,
                        D,
                        queue_num=swq(),
                        single_packet=True,
                    )

        # ============== readback: out_nm = dinv*(acc0+acc1+g_own) + brow ===
        def readback(acc_pair, g_own_list, brow, store_out, prelu_a, dst_big):
            for s in range(nch):
                r0 = s * CHN
                rows = min(CHN, shard - r0)
                ntile = _ceil(rows, P)
                at0 = xin.tile([P, CH, P], BF16, tag="at0")
                src = acc_pair[0][r0 : r0 + ntile * P, :].rearrange(
                    "(t p) f -> p t f", p=P
                )
                nc.sync.dma_start(out=at0[:, :ntile, :], in_=src)
                at1 = xin.tile([P, CH, P], BF16, tag="at1")
                src = acc_pair[1][r0 : r0 + ntile * P, :].rearrange(
                    "(t p) f -> p t f", p=P
                )
                nc.sync.dma_start(out=at1[:, :ntile, :], in_=src)
                gt = load_chunked_rows(g_own_list, chk, r0, ntile, xin, "gt")

                sm = work.tile([P, CH, P], F32, tag="sm")
                nc.vector.tensor_tensor(
                    out=sm[:, :ntile, :],
                    in0=at0[:, :ntile, :],
                    in1=at1[:, :ntile, :],
                    op=mybir.AluOpType.add,
                )
                nc.vector.tensor_tensor(
                    out=sm[:, :ntile, :],
                    in0=sm[:, :ntile, :],
                    in1=gt[:, :ntile, :],
                    op=mybir.AluOpType.add,
                )
                # * dinv (per-node = per (partition, tile)) via stride-0 f bcast
                dv_ap = bass.AP(
                    tensor=dinvc.tensor,
                    offset=dinvc.offset + s * CH,
                    ap=[list(dinvc.ap[0]), [1, ntile], [0, P]],
                )
                nc.vector.tensor_tensor(
                    out=sm[:, :ntile, :],
                    in0=sm[:, :ntile, :],
                    in1=dv_ap,
                    op=mybir.AluOpType.mult,
                )
                # + b row (replicated tile; bcast over the tile dim only)
                br_ap = bass.AP(
                    tensor=brow.tensor,
                    offset=brow.offset,
                    ap=[list(brow.ap[0]), [0, ntile], [1, P]],
                )
                ot = sm
                nc.vector.tensor_tensor(
                    out=ot[:, :ntile, :],
                    in0=sm[:, :ntile, :],
                    in1=br_ap,
                    op=mybir.AluOpType.add,
                )
                if store_out:
                    full, rem = divmod(rows, P)
                    if full:
                        dst = out[r0 : r0 + full * P, :].rearrange(
                            "(t p) f -> p t f", p=P
                        )
                        nc.sync.dma_start(out=dst, in_=ot[:, :full, :])
                    if rem:
                        nc.sync.dma_start(
                            out=out[r0 + full * P : r0 + rows, :],
                            in_=ot[:rem, full, :],
                        )
                else:
                    h2 = prelu_chunk(ot, prelu_a, ntile, BF16, work)
                    transpose_block(h2, ntile, dst_big, r0)

        # =================== schedule both convs ===========================
        for m in range(2):
            zero_acc(accs[0][m])
        for c in build_g(1, hT, w1b, g_shc[0]):
            issue_ag(0, c)
        # conv2 accumulators zero during conv1's collective/edge window
        for m in range(2):
            zero_acc(accs[1][m])
        sctr = [0]
        goff = 0
        for gg in range(NCHK):
            if caps[gg]:
                edge_group(g_full[0][gg], accs[0], gg, goff, sctr)
            goff += caps[gg]
        h2T = big.tile([P, shard_pad], BF16, tag="hbig")
        readback(accs[0], g_shc[0], b1row, False, a2c, h2T)
        if shard_pad > _rup(shard, P):
            nc.vector.memset(h2T[:, _rup(shard, P) :], 0.0)

        for c in build_g(2, h2T, w2b, g_shc[1]):
            issue_ag(1, c)
        sctr = [0]
        goff = 0
        for gg in range(NCHK):
            if caps[gg]:
                edge_group(g_full[1][gg], accs[1], gg, goff, sctr)
            goff += caps[gg]
        readback(accs[1], g_shc[1], b2row, True, None, None)

    # Spread SWDGE calls over the 4 rings: Tile assigns DMASW sem lanes
    # round-robin in Pool program order, so lane % 4 puts adjacent calls on
    # different rings (parallel drain) while each lane still maps to exactly
    # one ring (stable sem<->queue binding).
    from concourse.tile_sem_assignment import PROC_NAME_TO_IDX

    lane_of = {PROC_NAME_TO_IDX[f"DMASW{k}"]: k for k in range(8)}
    for inst in nc.inst_map.values():
        if isinstance(inst, (mybir.InstDMAGatherAnt, mybir.InstDMAScatterAddAnt)):
            proc = getattr(inst, "bass_scheduled_proc", None)
            if proc in lane_of:
                inst.queue_num = lane_of[proc] % 4

    nc.compile()
    return nc


# --------------------------------------------------------------------------
# Host wrapper
# --------------------------------------------------------------------------
def _prep_inputs(x, edge_index, a1, gamma, beta, W1, b1, a2, W2, b2):
    n = x.shape[0]
    shard = n // NC
    shard_pad = _rup(shard, NCHK * P)
    nt = shard_pad // P
    ei = np.asarray(edge_index).astype(np.int64)
    plan = EdgePlan(ei[0], ei[1], n)

    in_maps = []
    for c in range(NC):
        dv = plan.dinv[c * shard : (c + 1) * shard]
        dvp = np.zeros(shard_pad, dtype=np.float32)
        dvp[:shard] = dv
        dinv_c = np.ascontiguousarray(dvp.reshape(nt, P).T)  # [p,t]=dinv[t*128+p]
        in_maps.append(
            dict(
                x_sh=np.ascontiguousarray(x[c * shard : (c + 1) * shard]).astype(
                    np.float32
                ),
                w1=np.asarray(W1, dtype=np.float32),
                w2=np.asarray(W2, dtype=np.float32),
                b1r=np.asarray(b1, dtype=np.float32).reshape(1, D),
                b2r=np.asarray(b2, dtype=np.float32).reshape(1, D),
                gam=np.asarray(gamma, dtype=np.float32).reshape(D, 1),
                bet=np.asarray(beta, dtype=np.float32).reshape(D, 1),
                a1=np.asarray(a1, dtype=np.float32).reshape(1, 1),
                a2=np.asarray(a2, dtype=np.float32).reshape(1, 1),
                dinv_c=dinv_c,
                src_idx=plan.src16[c],
                dst_idx=plan.dst16[c],
            )
        )
    return plan, in_maps


_PROG_CACHE = {}


def kernel(x, edge_index, a1, gamma, beta, W1, b1, a2, W2, b2, _trace=False):
    x = np.asarray(x)
    n = x.shape[0]
    plan, in_maps = _prep_inputs(
        x, edge_index, a1, gamma, beta, W1, b1, a2, W2, b2
    )
    key = (n, tuple(plan.caps), tuple(tuple(r) for r in plan.rounds))
    if key not in _PROG_CACHE:
        _PROG_CACHE[key] = build_program(n, plan.caps, plan.rounds)
    nc = _PROG_CACHE[key]
    res = run_bass_kernel_spmd(
        nc, in_maps, core_ids=list(range(NC)), trace=_trace
    )
    outs = [res.results[c]["out"] for c in range(NC)]
    full = np.concatenate(outs, axis=0).astype(np.float32)
    kernel._last_exec_ns = res.exec_time_ns
    return full


# revision 14
# speedup vs baseline: 1.1897x; 1.0349x over previous
"""Trainium2 Bass kernel for nn_MeinBlock (2-layer GCN w/ PReLU + BatchNorm).

Computation (reference):
    h = PReLU(x, a1); h = BN(h, gamma, beta)
    h = GCNConv(h, W1, b1, edges); h = PReLU(h, a2)
    out = GCNConv(h, W2, b2, edges)

GCNConv(h) = dinv * segsum_dst(g[src]) + g*dinv + b   where
    g = dinv * (h @ W),  dinv = deg^-1/2, deg = 1 + indegree.
(The self-loop term (h@W)/deg equals g*dinv.)

Distribution: nodes sharded 8 ways (dst-partitioned edges per the hint).
Each core builds its g-shard in 8 row-chunks of 3200 (g_shc tensors); 8
chunked AllGathers (one per row-chunk) each produce a 25600-row bf16
gather table, so edge processing for chunk c starts as soon as collective
c lands -- the collective pipeline overlaps the gather/scatter pipeline.
Edges are grouped by the source row-chunk c = (src % shard) // 3200 and
dma_gather fetches messages (row index = src_shard*3200 + offset fits
int16), dma_scatter_add (CCE) accumulates into per-core DRAM accumulators.
Duplicate destinations within one scatter call would race in the SDMA CCE,
so edges are split into occurrence rounds (round r = r-th edge of its dst
within its chunk group).  Each round is further split into BANDS by fixed
dst ranges; a band's scatter writes only acc[d0:d1] (indices rebased to
the band base), so Tile's subregion dep tracking serializes only truly
overlapping calls: bands of one round run concurrently, successive rounds
pipeline band-by-band, and two accumulators alternate for extra slack.
Pad slots use index -1 (skipped by the SWDGE ucode, no descriptors).
SWDGE calls go to rings round-robin (queue = DMASW lane % 4) so all four
rings drain concurrently instead of head-of-line blocking on one ring.
BN batch stats are combined with a tiny AllReduce.
"""

import os
import sys
from contextlib import ExitStack

import numpy as np

sys.path.insert(0, "/opt/trn_rl_repo")

from concourse import bacc, bass, mybir, tile  # noqa: E402
from concourse import bass_utils as _bu  # noqa: E402
from concourse.bass_utils import run_bass_kernel_spmd  # noqa: E402
from concourse.masks import make_identity  # noqa: E402

# The image's antenv lacks axon_hooks; register the NTFF profile hook
# ourselves so trace=True can report HW exec time. Harmless if it fails.
def _install_ntff_hook():
    import types

    try:
        import antenv.axon_hooks  # noqa: F401
        return
    except ImportError:
        pass
    try:
        import antenv
        from trn_agent_boot.trn_boot import _ntff_profile_via_ctypes

        hook = _ntff_profile_via_ctypes("/opt/axon/libaxon_pjrt.so")
        mod = types.ModuleType("antenv.axon_hooks")
        mod.get_axon_ntff_profile_hook = lambda: hook
        mod.set_axon_ntff_profile_hook = lambda h: None
        sys.modules["antenv.axon_hooks"] = mod
        antenv.axon_hooks = mod
    except Exception:
        pass


_install_ntff_hook()
_bu.upload_artifacts = lambda tmpdir: tmpdir  # no artifact bucket here

F32 = mybir.dt.float32
BF16 = mybir.dt.bfloat16
I16 = mybir.dt.int16

P = 128          # partitions
D = 128          # feature dim
NC = 8           # cores
NCHK = 8         # collective row-chunks per shard
EPS = 1e-5
DUMMYROWS = 128  # scatter pad target rows appended to each accumulator
GCH = 512        # indices per SWDGE call (ring holds 1024 descs -> 2 calls)


def _ceil(a, b):
    return -(-a // b)


def _rup(a, b):
    return _ceil(a, b) * b


# --------------------------------------------------------------------------
# Host-side edge plan (pure index manipulation = the sharding step)
# --------------------------------------------------------------------------
class EdgePlan:
    """Per-core gather/scatter index tensors + static layout metadata.

    Edges of core c (dst in its shard) are grouped by the SOURCE row-chunk
    g = (src % shard) // chk (chk = shard_pad // NCHK rows per collective
    chunk).  Gather table for group g is the AllGather of every shard's
    rows [g*chk, (g+1)*chk): table row = src_shard * chk + (src_local -
    g*chk) < NC*chk = shard_pad <= 32767 (int16).
    """

    def __init__(self, src, dst, n_nodes):
        shard = n_nodes // NC
        self.shard = shard
        shard_pad = _rup(shard, NCHK * P)
        chk = shard_pad // NCHK
        assert chk * NCHK == shard_pad and NC * chk < 32768
        self.chk = chk
        deg = np.bincount(dst, minlength=n_nodes).astype(np.float64) + 1.0
        self.dinv = (1.0 / np.sqrt(deg)).astype(np.float32)

        per_core = []  # (g_arr, r_arr, tblrow, dst_local) sorted by (g, r, dst)
        maxlen = np.zeros((NCHK, 64), dtype=np.int64)  # [g, r] -> max count
        maxr = np.zeros(NCHK, dtype=np.int64)
        for c in range(NC):
            m = (dst // shard) == c
            es, ed = src[m], dst[m]
            dl = (ed - c * shard).astype(np.int64)
            j = (es // shard).astype(np.int64)
            sl = (es - j * shard).astype(np.int64)   # local row in src shard
            g = sl // chk                            # source row-chunk group
            row = j * chk + (sl - g * chk)           # row in group-g table
            # occurrence round of each edge's dst within its group g
            order = np.lexsort((dl, g))
            g_s, dl_s, row_s = g[order], dl[order], row[order]
            key = g_s * shard + dl_s
            first = np.ones(len(key), dtype=bool)
            first[1:] = key[1:] != key[:-1]
            run_id = np.cumsum(first) - 1
            run_start = np.flatnonzero(first)
            r = np.arange(len(key)) - run_start[run_id]
            # final order: by (g, r), stable -> dst-sorted within each round
            order2 = np.lexsort((r, g_s))
            g_f, r_f = g_s[order2], r[order2]
            per_core.append((g_f, r_f, row_s[order2], dl_s[order2]))
            for gg in range(NCHK):
                mg = g_f == gg
                if mg.any():
                    rg = r_f[mg]
                    maxr[gg] = max(maxr[gg], rg.max() + 1)
                    cnt = np.bincount(rg)
                    maxlen[gg, : len(cnt)] = np.maximum(maxlen[gg, : len(cnt)], cnt)

        # static banded layout shared by all cores: each (group, round) is
        # split into bands of fixed dst ranges; per-band padded length is the
        # max count over cores (rounded to 128).
        per_core_cnt = []  # per core: {(g, r, b): count}
        bands_meta = []    # temp: per (g, r): nb, [d0, d1) list
        for gg in range(NCHK):
            rl = []
            for rr in range(int(maxr[gg])):
                nb = max(1, min(32, int(maxlen[gg, rr]) // 448))
                bsz = _ceil(shard, nb)
                rl.append([(b * bsz, min(shard, (b + 1) * bsz)) for b in range(nb)])
            bands_meta.append(rl)
        for c in range(NC):
            g_f, r_f, row_f, dl_f = per_core[c]
            cnts = {}
            for gg in range(NCHK):
                mg = g_f == gg
                rg, dg = r_f[mg], dl_f[mg]
                for rr, blist in enumerate(bands_meta[gg]):
                    mr = rg == rr
                    db = dg[mr]
                    bsz = _ceil(shard, len(blist))
                    bc = np.bincount(db // bsz, minlength=len(blist))
                    for b in range(len(blist)):
                        cnts[(gg, rr, b)] = int(bc[b])
            per_core_cnt.append(cnts)

        # self.bands[g] = list of (offset_in_group, padded_len, d0, d1)
        self.bands = []
        self.caps = []
        off = 0
        for gg in range(NCHK):
            bl = []
            goff = off
            for rr, blist in enumerate(bands_meta[gg]):
                for b, (d0, d1) in enumerate(blist):
                    mx = max(per_core_cnt[c][(gg, rr, b)] for c in range(NC))
                    if mx == 0:
                        continue
                    plen = int(_rup(mx, P))
                    bl.append((off - goff, plen, d0, d1))
                    off += plen
            self.bands.append(bl)
            self.caps.append(off - goff)
        self.sumcap = off

        # fill per-core index arrays.  Pad slots gather one of the P zero
        # rows appended to each table (row tbl+k) and scatter-add the zeros
        # onto rows of the band that carry NO real edge on this core, so the
        # zero-adds can never race a real accumulation.
        tbl = NC * chk
        self.tbl = tbl
        self.src16 = np.zeros((NC, P, self.sumcap // 16), dtype=np.int16)
        self.dst16 = np.zeros((NC, P, self.sumcap // 16), dtype=np.int16)
        for c in range(NC):
            g_f, r_f, row_f, dl_f = per_core[c]
            sarr = np.zeros(self.sumcap, dtype=np.int16)
            darr = np.zeros(self.sumcap, dtype=np.int16)
            goff = 0
            for gg in range(NCHK):
                mg = g_f == gg
                rg, dg, rowg = r_f[mg], dl_f[mg], row_f[mg]
                bi = 0
                for rr, blist in enumerate(bands_meta[gg]):
                    mr = rg == rr
                    db, rowb = dg[mr], rowg[mr]
                    bsz = _ceil(shard, len(blist))
                    bb = db // bsz
                    for b, (d0, d1) in enumerate(blist):
                        mx = max(per_core_cnt[cc][(gg, rr, b)] for cc in range(NC))
                        if mx == 0:
                            continue
                        boff, plen, bd0, bd1 = self.bands[gg][bi]
                        assert (bd0, bd1) == (d0, d1)
                        bi += 1
                        sel = bb == b
                        n = int(sel.sum())
                        assert n <= plen
                        base = goff + boff
                        sarr[base : base + n] = rowb[sel]
                        darr[base : base + n] = db[sel] - d0
                        npad = plen - n
                        if npad:
                            used = np.zeros(d1 - d0, dtype=bool)
                            used[db[sel] - d0] = True
                            free = np.flatnonzero(~used)
                            assert len(free) >= min(npad, P), (gg, rr, b)
                            sarr[base + n : base + plen] = tbl + (
                                np.arange(npad) % P
                            )
                            darr[base + n : base + plen] = free[
                                np.arange(npad) % len(free)
                            ]
                goff += self.caps[gg]
            w = sarr.reshape(-1, 16).T
            self.src16[c] = np.tile(w, (8, 1))
            w = darr.reshape(-1, 16).T
            self.dst16[c] = np.tile(w, (8, 1))


# --------------------------------------------------------------------------
# Device program
# --------------------------------------------------------------------------
def build_program(n_nodes, caps, bands):
    """One SPMD program for all 8 cores. caps/bands = static edge layout."""
    shard = n_nodes // NC
    shard_pad = _rup(shard, NCHK * P)
    nt = shard_pad // P                 # 128-node tiles per shard
    chk = shard_pad // NCHK             # rows per collective chunk
    tbl = NC * chk                      # rows per gather table
    sumcap = sum(caps)
    accrows = _rup(shard, P)

    nc = bacc.Bacc(
        "TRN2",
        target_bir_lowering=False,
        debug=False,
        num_devices=NC,
        num_swdge_queues=4,
    )

    x_sh = nc.declare_dram_parameter("x_sh", [shard, D], F32, isOutput=False)
    w1 = nc.declare_dram_parameter("w1", [D, D], F32, isOutput=False)
    w2 = nc.declare_dram_parameter("w2", [D, D], F32, isOutput=False)
    b1r = nc.declare_dram_parameter("b1r", [1, D], F32, isOutput=False)
    b2r = nc.declare_dram_parameter("b2r", [1, D], F32, isOutput=False)
    gam = nc.declare_dram_parameter("gam", [D, 1], F32, isOutput=False)
    bet = nc.declare_dram_parameter("bet", [D, 1], F32, isOutput=False)
    a1 = nc.declare_dram_parameter("a1", [1, 1], F32, isOutput=False)
    a2 = nc.declare_dram_parameter("a2", [1, 1], F32, isOutput=False)
    dinv_c = nc.declare_dram_parameter("dinv_c", [P, nt], F32, isOutput=False)
    src_idx = nc.declare_dram_parameter("src_idx", [P, sumcap // 16], I16, isOutput=False)
    dst_idx = nc.declare_dram_parameter("dst_idx", [P, sumcap // 16], I16, isOutput=False)
    out = nc.declare_dram_parameter("out", [shard, D], F32, isOutput=True)

    # per-conv: 8 local g row-chunks + 8 allgathered tables
    g_shc = [
        [nc.dram_tensor(f"g{i}_shc{c}", [chk, D], BF16) for c in range(NCHK)]
        for i in (1, 2)
    ]
    # tables carry P extra zero rows at [tbl, tbl+P) as gather targets for
    # pad slots (scatter-adds of zero onto edge-free rows)
    g_full = [
        [
            nc.dram_tensor(f"g{i}_full{c}", [tbl + P, D], BF16, addr_space="Shared")
            for c in range(NCHK)
        ]
        for i in (1, 2)
    ]
    accs = [
        [nc.dram_tensor(f"acc{i}_{m}", [accrows, D], BF16) for m in range(2)]
        for i in (0, 1)
    ]
    bn_in = nc.dram_tensor("bn_in", [P, 2], F32)
    bn_out = nc.dram_tensor("bn_out", [P, 2], F32, addr_space="Shared")

    CH = 8           # 128-node tiles per big chunk
    CHN = CH * P     # nodes per big chunk (1024)

    with tile.TileContext(nc) as tc, ExitStack() as ctx:
        singles = ctx.enter_context(tc.tile_pool(name="singles", bufs=1))
        big = ctx.enter_context(tc.tile_pool(name="big", bufs=1))
        xin = ctx.enter_context(tc.tile_pool(name="xin", bufs=2))
        work = ctx.enter_context(tc.tile_pool(name="work", bufs=2))
        stream = ctx.enter_context(tc.tile_pool(name="stream", bufs=3))
        gout = ctx.enter_context(tc.tile_pool(name="gout", bufs=3))
        msgs_tp = ctx.enter_context(tc.tile_pool(name="msgs", bufs=8))
        mm_tp = ctx.enter_context(tc.tile_pool(name="mm", bufs=2, space="PSUM"))
        tp_tp = ctx.enter_context(tc.tile_pool(name="tp", bufs=4, space="PSUM"))
        stat_tp = ctx.enter_context(tc.tile_pool(name="stat", bufs=1))

        # ---- constants -------------------------------------------------
        idbf = singles.tile([P, P], BF16)
        make_identity(nc, idbf[:])
        a1c = singles.tile([P, 1], F32)
        nc.sync.dma_start(out=a1c[:], in_=a1[:].to_broadcast([P, 1]))
        a2c = singles.tile([P, 1], F32)
        nc.sync.dma_start(out=a2c[:], in_=a2[:].to_broadcast([P, 1]))
        b1row = singles.tile([P, D], F32)
        nc.sync.dma_start(out=b1row[:], in_=b1r[:].to_broadcast([P, D]))
        b2row = singles.tile([P, D], F32)
        nc.sync.dma_start(out=b2row[:], in_=b2r[:].to_broadcast([P, D]))
        gamc = singles.tile([P, 1], F32)
        nc.sync.dma_start(out=gamc[:], in_=gam[:])
        betc = singles.tile([P, 1], F32)
        nc.sync.dma_start(out=betc[:], in_=bet[:])
        dinvc = singles.tile([P, nt], F32)
        nc.sync.dma_start(out=dinvc[:], in_=dinv_c[:])
        w1f = singles.tile([P, D], F32)
        nc.sync.dma_start(out=w1f[:], in_=w1[:])
        w1b = singles.tile([P, D], BF16)
        nc.vector.tensor_copy(w1b[:], w1f[:])
        w2f = singles.tile([P, D], F32)
        nc.sync.dma_start(out=w2f[:], in_=w2[:])
        w2b = singles.tile([P, D], BF16)
        nc.vector.tensor_copy(w2b[:], w2f[:])
        sidx = singles.tile([P, sumcap // 16], I16)
        nc.sync.dma_start(out=sidx[:], in_=src_idx[:])
        didx = singles.tile([P, sumcap // 16], I16)
        nc.sync.dma_start(out=didx[:], in_=dst_idx[:])
        zt = singles.tile([P, CH, P], BF16)
        nc.vector.memset(zt[:], 0.0)

        hT = big.tile([P, shard_pad], BF16, tag="hbig")

        def zero_acc(acc):
            zn = CH * P
            for s in range(_ceil(accrows, zn)):
                r0 = s * zn
                rows = min(zn, accrows - r0)
                full, rem = divmod(rows, P)
                if full:
                    dst = acc[r0 : r0 + full * P, :].rearrange(
                        "(t p) f -> p t f", p=P
                    )
                    nc.sync.dma_start(out=dst, in_=zt[:, :full, :])
                if rem:
                    dst2 = acc[r0 + full * P : r0 + rows, :]
                    nc.sync.dma_start(out=dst2, in_=zt[:rem, 0, :])

        def load_node_chunk(dram, r0, rows, dtype, pool):
            """DRAM rows [r0, r0+rows) -> SBUF [128, ceil(rows/128), 128]."""
            full, rem = divmod(rows, P)
            t = pool.tile([P, CH, P], dtype, tag="ld")
            if full:
                src = dram[r0 : r0 + full * P, :].rearrange("(t p) f -> p t f", p=P)
                nc.sync.dma_start(out=t[:, :full, :], in_=src)
            if rem:
                nc.vector.memset(t[:, full, :], 0.0)
                nc.sync.dma_start(
                    out=t[:rem, full, :], in_=dram[r0 + full * P : r0 + rows, :]
                )
            return t

        def load_chunked_rows(tensors, rowsz, r0, ntile, pool, tag):
            """Rows [r0, r0+ntile*128) from a list of [rowsz, D] DRAM tensors
            (concatenated view) -> SBUF [128, ntile, 128]."""
            t = pool.tile([P, CH, P], BF16, tag=tag)
            a = r0
            while a < r0 + ntile * P:
                c = a // rowsz
                b = min(r0 + ntile * P, (c + 1) * rowsz)
                src = tensors[c][a - c * rowsz : b - c * rowsz, :].rearrange(
                    "(t p) f -> p t f", p=P
                )
                nc.sync.dma_start(
                    out=t[:, (a - r0) // P : (b - r0) // P, :], in_=src
                )
                a = b
            return t

        def transpose_block(src_bf16, ntile, dst_big, col0):
            """node-major [128, ntile, 128] -> dst_big[:, col0 : col0+128*ntile]."""
            for k in range(ntile):
                pt = tp_tp.tile([P, P], BF16, tag="tp")
                nc.tensor.transpose(out=pt[:], in_=src_bf16[:, k, :], identity=idbf[:])
                nc.any.tensor_copy(
                    out=dst_big[:, col0 + k * P : col0 + (k + 1) * P], in_=pt[:]
                )

        def prelu_chunk(x_f32, ac, ntile, out_dtype, pool):
            """max(x, a*x) on [128, ntile, 128]."""
            ax = pool.tile([P, CH, P], F32, tag="ax")
            nc.vector.tensor_scalar_mul(ax[:, :ntile, :], x_f32[:, :ntile, :], ac[:, :1])
            h = pool.tile([P, CH, P], out_dtype, tag="h")
            nc.vector.tensor_tensor(
                out=h[:, :ntile, :],
                in0=x_f32[:, :ntile, :],
                in1=ax[:, :ntile, :],
                op=mybir.AluOpType.max,
            )
            return h

        # ================= conv1 phase A: x -> hT (bf16, feature-major) ====
        nch = _ceil(shard, CHN)
        for s in range(nch):
            r0 = s * CHN
            rows = min(CHN, shard - r0)
            ntile = _ceil(rows, P)
            xt = load_node_chunk(x_sh, r0, rows, F32, xin)
            h = prelu_chunk(xt, a1c, ntile, BF16, work)
            transpose_block(h, ntile, hT, r0)
        if shard_pad > _rup(shard, P):
            nc.vector.memset(hT[:, _rup(shard, P) :], 0.0)

        # ================= BN stats + allreduce ============================
        q = 500 if shard % 500 == 0 else int(np.gcd(shard, 512))
        while shard % q or q > 512:
            q -= 1
        sg = shard // q
        stats = stat_tp.tile([P, sg, 6], F32)
        hT3 = hT[:, :shard].rearrange("p (s q) -> p s q", q=q)
        for i in range(sg):
            nc.vector.bn_stats(out=stats[:, i, :], in_=hT3[:, i, :])
        mv = stat_tp.tile([P, 2], F32)
        nc.vector.bn_aggr(out=mv[:], in_=stats[:])
        # allreduce (mean/8, (var+mean^2)/8)
        ar = stat_tp.tile([P, 2], F32)
        nc.vector.tensor_tensor(
            out=ar[:, 1:2], in0=mv[:, 0:1], in1=mv[:, 0:1], op=mybir.AluOpType.mult
        )
        nc.vector.tensor_tensor(
            out=ar[:, 1:2], in0=ar[:, 1:2], in1=mv[:, 1:2], op=mybir.AluOpType.add
        )
        nc.vector.tensor_scalar_mul(ar[:, 1:2], ar[:, 1:2], 1.0 / NC)
        nc.vector.tensor_scalar_mul(ar[:, 0:1], mv[:, 0:1], 1.0 / NC)
        nc.sync.dma_start(out=bn_in[:], in_=ar[:])
        nc.gpsimd.collective_compute(
            "AllReduce",
            mybir.AluOpType.add,
            replica_groups=[list(range(NC))],
            ins=[bn_in[:]],
            outs=[bn_out[:]],
        )
        st = stat_tp.tile([P, 2], F32)
        nc.sync.dma_start(out=st[:], in_=bn_out[:])
        var = stat_tp.tile([P, 1], F32)
        nc.vector.tensor_tensor(
            out=var[:], in0=st[:, 0:1], in1=st[:, 0:1], op=mybir.AluOpType.mult
        )
        nc.vector.tensor_tensor(
            out=var[:], in0=st[:, 1:2], in1=var[:], op=mybir.AluOpType.subtract
        )
        epst = stat_tp.tile([P, 1], F32)
        nc.vector.memset(epst[:], EPS)
        rstd = stat_tp.tile([P, 1], F32)
        nc.scalar.activation(
            out=rstd[:],
            in_=var[:],
            func=mybir.ActivationFunctionType.Sqrt,
            bias=epst[:],
        )
        nc.vector.reciprocal(out=rstd[:], in_=rstd[:])
        scol = stat_tp.tile([P, 1], F32)
        nc.vector.tensor_tensor(
            out=scol[:], in0=gamc[:], in1=rstd[:], op=mybir.AluOpType.mult
        )
        tcol = stat_tp.tile([P, 1], F32)
        nc.vector.tensor_tensor(
            out=tcol[:], in0=st[:, 0:1], in1=scol[:], op=mybir.AluOpType.mult
        )
        nc.vector.tensor_tensor(
            out=tcol[:], in0=betc[:], in1=tcol[:], op=mybir.AluOpType.subtract
        )

        # ====== shared: hT -> g (normalize? -> matmul -> T -> dinv -> store)
        # Chunked-store into the 8 per-collective g_shc tensors; the matching
        # AllGather is issued as soon as its chunk's rows are all stored, so
        # collectives pipeline with the rest of build_g (and then with the
        # edge phase).
        MC = 512  # nodes per matmul chunk

        def build_g(conv, src_big, wts, g_dst_list):
            nmc = _ceil(shard_pad, MC)
            next_ag = 0
            for m in range(nmc):
                c0 = m * MC
                cols = min(MC, shard_pad - c0)
                if conv == 1:
                    nh = stream.tile([P, MC], BF16, tag="nh")
                    nc.scalar.activation(
                        out=nh[:, :cols],
                        in_=src_big[:, c0 : c0 + cols],
                        func=mybir.ActivationFunctionType.Identity,
                        bias=tcol[:],
                        scale=scol[:],
                    )
                    rhs = nh[:, :cols]
                else:
                    rhs = src_big[:, c0 : c0 + cols]
                mm = mm_tp.tile([P, MC], F32, tag="mm")
                nc.tensor.matmul(
                    out=mm[:, :cols], lhsT=wts[:], rhs=rhs, start=True, stop=True
                )
                gT = stream.tile([P, MC], BF16, tag="gT")
                nc.any.tensor_copy(out=gT[:, :cols], in_=mm[:, :cols])
                stg = gout.tile([P, MC // P, P], BF16, tag="stg")
                for k in range(_ceil(cols, P)):
                    pt = tp_tp.tile([P, P], BF16, tag="tp")
                    nc.tensor.transpose(
                        out=pt[:], in_=gT[:, k * P : (k + 1) * P], identity=idbf[:]
                    )
                    nc.any.tensor_copy(out=stg[:, k, :], in_=pt[:])
                # * dinv (per-node = per (partition, tile)) via stride-0 f bcast
                ntile = _ceil(cols, P)
                dv_ap = bass.AP(
                    tensor=dinvc.tensor,
                    offset=dinvc.offset + c0 // P,
                    ap=[list(dinvc.ap[0]), [1, ntile], [0, P]],
                )
                nc.vector.tensor_tensor(
                    out=stg[:, :ntile, :],
                    in0=stg[:, :ntile, :],
                    in1=dv_ap,
                    op=mybir.AluOpType.mult,
                )
                # store rows [c0, c0+cols) split at chunk boundaries
                a = c0
                while a < c0 + cols:
                    cc = a // chk
                    b = min(c0 + cols, (cc + 1) * chk)
                    dst = g_dst_list[cc][a - cc * chk : b - cc * chk, :].rearrange(
                        "(t p) f -> p t f", p=P
                    )
                    nc.sync.dma_start(
                        out=dst, in_=stg[:, (a - c0) // P : (b - c0) // P, :]
                    )
                    a = b
                # issue any AllGather whose input chunk is now fully stored
                while next_ag < NCHK and (next_ag + 1) * chk <= c0 + cols:
                    yield next_ag
                    next_ag += 1
            while next_ag < NCHK:
                yield next_ag
                next_ag += 1

        def issue_ag(conv_i, c):
            nc.gpsimd.collective_compute(
                "AllGather",
                mybir.AluOpType.bypass,
                replica_groups=[list(range(NC))],
                ins=[g_shc[conv_i][c][:]],
                outs=[g_full[conv_i][c][0:tbl, :]],
            )

        # ============== edge phase: gather + banded scatter rounds =========
        # One gather + one scatter per band.  The scatter's out_ap is the
        # band's true dst row range acc[d0:d1] (indices rebased to d0), so
        # Tile's subregion dep tracking orders only genuinely overlapping
        # calls.  Pad indices are -1 (skipped, no descriptors).  queue_num is
        # rewritten post-scheduling to DMASW lane % 4 so adjacent calls land
        # on different rings and all four rings drain in parallel.
        def swq():
            return 0  # rewritten post-scheduling from the assigned DMASW lane

        def edge_group(g_full_t, acc_pair, gg, goff, sctr):
            for boff, plen, d0, d1 in bands[gg]:
                msgs = msgs_tp.tile([P, 8, P], BF16, tag="msgs")
                assert plen <= 8 * P
                nc.gpsimd.dma_gather(
                    msgs[:, : plen // P, :],
                    g_full_t[:],
                    sidx[:, (goff + boff) // 16 : (goff + boff + plen) // 16],
                    plen,
                    plen,
                    D,
                    queue_num=swq(),
                    single_packet=True,
                )
                acc = acc_pair[sctr[0] % len(acc_pair)]
                sctr[0] += 1
                nc.gpsimd.dma_scatter_add(
                    acc[d0:d1, :],
                    msgs[:, : plen // P, :],
                    didx[:, (goff + boff) // 16 : (goff + boff + plen) // 16],
                    plen,
                    plen,
                    D,
                    queue_num=swq(),
                    single_packet=True,
                )

        # ============== readback: out_nm = dinv*(acc0+acc1+g_own) + brow ===
        def readback(acc_pair, g_own_list, brow, store_out, prelu_a, dst_big):
            for s in range(nch):
                r0 = s * CHN
                rows = min(CHN, shard - r0)
                ntile = _ceil(rows, P)
                at0 = xin.tile([P, CH, P], BF16, tag="at0")
                src = acc_pair[0][r0 : r0 + ntile * P, :].rearrange(
                    "(t p) f -> p t f", p=P
                )
                nc.sync.dma_start(out=at0[:, :ntile, :], in_=src)
                at1 = xin.tile([P, CH, P], BF16, tag="at1")
                src = acc_pair[1][r0 : r0 + ntile * P, :].rearrange(
                    "(t p) f -> p t f", p=P
                )
                nc.sync.dma_start(out=at1[:, :ntile, :], in_=src)
                gt = load_chunked_rows(g_own_list, chk, r0, ntile, xin, "gt")

                sm = work.tile([P, CH, P], F32, tag="sm")
                nc.vector.tensor_tensor(
                    out=sm[:, :ntile, :],
                    in0=at0[:, :ntile, :],
                    in1=at1[:, :ntile, :],
                    op=mybir.AluOpType.add,
                )
                nc.vector.tensor_tensor(
                    out=sm[:, :ntile, :],
                    in0=sm[:, :ntile, :],
                    in1=gt[:, :ntile, :],
                    op=mybir.AluOpType.add,
                )
                # * dinv (per-node = per (partition, tile)) via stride-0 f bcast
                dv_ap = bass.AP(
                    tensor=dinvc.tensor,
                    offset=dinvc.offset + s * CH,
                    ap=[list(dinvc.ap[0]), [1, ntile], [0, P]],
                )
                nc.vector.tensor_tensor(
                    out=sm[:, :ntile, :],
                    in0=sm[:, :ntile, :],
                    in1=dv_ap,
                    op=mybir.AluOpType.mult,
                )
                # + b row (replicated tile; bcast over the tile dim only)
                br_ap = bass.AP(
                    tensor=brow.tensor,
                    offset=brow.offset,
                    ap=[list(brow.ap[0]), [0, ntile], [1, P]],
                )
                ot = sm
                nc.vector.tensor_tensor(
                    out=ot[:, :ntile, :],
                    in0=sm[:, :ntile, :],
                    in1=br_ap,
                    op=mybir.AluOpType.add,
                )
                if store_out:
                    full, rem = divmod(rows, P)
                    if full:
                        dst = out[r0 : r0 + full * P, :].rearrange(
                            "(t p) f -> p t f", p=P
                        )
                        nc.sync.dma_start(out=dst, in_=ot[:, :full, :])
                    if rem:
                        nc.sync.dma_start(
                            out=out[r0 + full * P : r0 + rows, :],
                            in_=ot[:rem, full, :],
                        )
                else:
                    h2 = prelu_chunk(ot, prelu_a, ntile, BF16, work)
                    transpose_block(h2, ntile, dst_big, r0)

        # =================== schedule both convs ===========================
        for i in range(2):
            for c in range(NCHK):
                nc.sync.dma_start(out=g_full[i][c][tbl : tbl + P, :], in_=zt[:, 0, :])
        for m in range(2):
            zero_acc(accs[0][m])
        for c in build_g(1, hT, w1b, g_shc[0]):
            issue_ag(0, c)
        # conv2 accumulators zero during conv1's collective/edge window
        for m in range(2):
            zero_acc(accs[1][m])
        sctr = [0]
        goff = 0
        for gg in range(NCHK):
            if caps[gg]:
                edge_group(g_full[0][gg], accs[0], gg, goff, sctr)
            goff += caps[gg]
        h2T = big.tile([P, shard_pad], BF16, tag="hbig")
        readback(accs[0], g_shc[0], b1row, False, a2c, h2T)
        if shard_pad > _rup(shard, P):
            nc.vector.memset(h2T[:, _rup(shard, P) :], 0.0)

        for c in build_g(2, h2T, w2b, g_shc[1]):
            issue_ag(1, c)
        sctr = [0]
        goff = 0
        for gg in range(NCHK):
            if caps[gg]:
                edge_group(g_full[1][gg], accs[1], gg, goff, sctr)
            goff += caps[gg]
        readback(accs[1], g_shc[1], b2row, True, None, None)

    # Spread SWDGE calls over the 4 rings: Tile assigns DMASW sem lanes
    # round-robin in Pool program order, so lane % 4 puts adjacent calls on
    # different rings (parallel drain) while each lane still maps to exactly
    # one ring (stable sem<->queue binding).
    from concourse.tile_sem_assignment import PROC_NAME_TO_IDX

    lane_of = {PROC_NAME_TO_IDX[f"DMASW{k}"]: k for k in range(8)}
    for inst in nc.inst_map.values():
        if isinstance(inst, (mybir.InstDMAGatherAnt, mybir.InstDMAScatterAddAnt)):
            proc = getattr(inst, "bass_scheduled_proc", None)
            if proc in lane_of:
                inst.queue_num = lane_of[proc] % 4

    nc.compile()
    return nc


# --------------------------------------------------------------------------
# Host wrapper
# --------------------------------------------------------------------------
def _prep_inputs(x, edge_index, a1, gamma, beta, W1, b1, a2, W2, b2):
    n = x.shape[0]
    shard = n // NC
    shard_pad = _rup(shard, NCHK * P)
    nt = shard_pad // P
    ei = np.asarray(edge_index).astype(np.int64)
    plan = EdgePlan(ei[0], ei[1], n)

    in_maps = []
    for c in range(NC):
        dv = plan.dinv[c * shard : (c + 1) * shard]
        dvp = np.zeros(shard_pad, dtype=np.float32)
        dvp[:shard] = dv
        dinv_c = np.ascontiguousarray(dvp.reshape(nt, P).T)  # [p,t]=dinv[t*128+p]
        in_maps.append(
            dict(
                x_sh=np.ascontiguousarray(x[c * shard : (c + 1) * shard]).astype(
                    np.float32
                ),
                w1=np.asarray(W1, dtype=np.float32),
                w2=np.asarray(W2, dtype=np.float32),
                b1r=np.asarray(b1, dtype=np.float32).reshape(1, D),
                b2r=np.asarray(b2, dtype=np.float32).reshape(1, D),
                gam=np.asarray(gamma, dtype=np.float32).reshape(D, 1),
                bet=np.asarray(beta, dtype=np.float32).reshape(D, 1),
                a1=np.asarray(a1, dtype=np.float32).reshape(1, 1),
                a2=np.asarray(a2, dtype=np.float32).reshape(1, 1),
                dinv_c=dinv_c,
                src_idx=plan.src16[c],
                dst_idx=plan.dst16[c],
            )
        )
    return plan, in_maps


_PROG_CACHE = {}


def kernel(x, edge_index, a1, gamma, beta, W1, b1, a2, W2, b2, _trace=False):
    x = np.asarray(x)
    n = x.shape[0]
    plan, in_maps = _prep_inputs(
        x, edge_index, a1, gamma, beta, W1, b1, a2, W2, b2
    )
    key = (n, tuple(plan.caps), tuple(tuple(r) for r in plan.bands))
    if key not in _PROG_CACHE:
        _PROG_CACHE[key] = build_program(n, plan.caps, plan.bands)
    nc = _PROG_CACHE[key]
    res = run_bass_kernel_spmd(
        nc, in_maps, core_ids=list(range(NC)), trace=_trace
    )
    outs = [res.results[c]["out"] for c in range(NC)]
    full = np.concatenate(outs, axis=0).astype(np.float32)
    kernel._last_exec_ns = res.exec_time_ns
    return full
